# revision 1
# baseline (speedup 1.0000x reference)
"""Trainium2 Bass kernel for nn_AttentionLayer (B=4, S=2048, D=H=512).

Reference computation:
    q = x@Wq + bq; k = x@Wk + bk; v = x@Wv + bv          # [B,S,H]
    qk = q @ k^T                                          # [B,S,S]
    dense = sigmoid(qk @ Wd + bd)                         # [B,S,H]
    vw = dense @ v^T                                      # [B,S,S]
    out = vw @ x                                          # [B,S,D]

Algebraic refactor (associativity): neither the [S,S] intermediates nor
q/k/v ever materialize:
    G1 = x^T @ Wd                  # [D,H']
    A1 = Wk^T @ G1 (+ outer(bk, colsum(Wd)))   # [H,H']  == k^T @ Wd
    X2 = x^T @ x                   # [D,D]
    A2 = Wv^T @ X2 (+ outer(bv, colsum(x)))    # [H,D]   == v^T @ x
    M1 = Wq @ A1                   # [D,H']
    b1 = A1^T @ bq + bd            # [H']
    denseT = sigmoid(M1^T @ x_own^T + b1)      # [H',R]
    out = denseT^T @ A2            # [R,D]

Sharding: 8 cores = (batch b, seq-half). SPMD graphs are identical across
cores; per-core row offsets are realized by rolling x[b] and Wd rows on the
host so each core's "own" rows are always rows 0:R. All contractions over
the sequence axis are invariant to a consistent roll of {x, Wd} rows.

x and the weights are cast to bf16 on the host (identical numerics to an
on-device cast, half the DMA bytes). Compute is bf16 matmuls with f32 PSUM
accumulation; the output is written f32.
"""

import numpy as np
import ml_dtypes

B, S, D, H = 4, 2048, 512, 512
P = 128
R = S // 2            # rows owned per core
NF = 512              # matmul moving free dim (one PSUM bank of f32)
T_TILES = S // P      # 16
D_TILES = D // P      # 4
H_TILES = H // P      # 4
R_TILES = R // P      # 8
N_CORES = 8

_NC = {}
LAST_RESULTS = None   # BassKernelResults of the most recent run


def _build_body(nc, tc, aps, repeat=1, zero_bias=False):
    import concourse.mybir as mybir
    from concourse.masks import make_identity
    from contextlib import ExitStack

    BF = mybir.dt.bfloat16
    F32 = mybir.dt.float32
    AF = mybir.ActivationFunctionType

    x_d, wqkT_d, wv_d, wd_d, bq_d, bd_d, aux_d, out_d = aps

    # chunked layouts: 4 tiles per DMA
    x_dr = x_d.rearrange("(c a p) d -> c p a d", p=P, a=4)
    wd_dr = wd_d.rearrange("(c a p) h -> c p a h", p=P, a=4)
    wqkT_dr = wqkT_d.rearrange("(i p) d -> p i d", p=P)
    wv_dr = wv_d.rearrange("(i p) h -> p i h", p=P)
    out_dr = out_d.rearrange("(c m p) d -> c p m d", p=P, m=4)

    ctx = ExitStack()
    big = ctx.enter_context(tc.tile_pool(name="big", bufs=1))
    const = ctx.enter_context(tc.tile_pool(name="const", bufs=1))
    psum_mm = ctx.enter_context(tc.tile_pool(name="psum_mm", bufs=4, space="PSUM"))
    psum_acc = ctx.enter_context(tc.tile_pool(name="psum_acc", bufs=3, space="PSUM"))
    psum_tr = ctx.enter_context(tc.tile_pool(name="psum_tr", bufs=1, space="PSUM"))

    # constants (loaded once)
    ident = const.tile([P, P], BF, name="ident")
    make_identity(nc, ident)

    if not zero_bias:
        bd_sb = const.tile([P, H_TILES], F32, name="bd_sb")
        bq_col = const.tile([P, H_TILES], BF, name="bq_col")
        aux_rows = [const.tile([1, H], BF, name=nm)
                    for nm in ("w2_row", "swd_row", "bv_row", "sx_row")]
        w2_row, swd_row, bv_row, sx_row = aux_rows
        b1_sb = const.tile([P, H_TILES], F32, name="b1_sb")

    for _rep in range(repeat):
        # ---- input DMAs, emitted in order of first use ----
        # x and Wd chunks interleave across the two HWDGE queues so G1/X2 can
        # start as early as possible.
        x_bf = big.tile([P, T_TILES, D], BF, name="x_bf")
        wd_bf = big.tile([P, T_TILES, H], BF, name="wd_bf")
        # first chunk split in half so the first matmuls can start sooner
        nc.sync.dma_start(x_bf[:, 0:2, :], x_dr[0][:, 0:2, :])
        nc.scalar.dma_start(wd_bf[:, 0:2, :], wd_dr[0][:, 0:2, :])
        nc.sync.dma_start(x_bf[:, 2:4, :], x_dr[0][:, 2:4, :])
        nc.scalar.dma_start(wd_bf[:, 2:4, :], wd_dr[0][:, 2:4, :])
        for c in range(1, T_TILES // 4):
            nc.sync.dma_start(x_bf[:, 4 * c:4 * (c + 1), :], x_dr[c])
            nc.scalar.dma_start(wd_bf[:, 4 * c:4 * (c + 1), :], wd_dr[c])

        # transpose own x rows -> xT_own [d, s] via DMA-transpose
        xT_own = big.tile([P, D_TILES, R], BF, name="xT_own")
        for i in range(D_TILES):
            nc.sync.dma_start_transpose(xT_own[:, i, :], x_d[0:R, i * P:(i + 1) * P])

        # weights (needed after G1/X2 complete)
        wqkT_bf = big.tile([P, D_TILES, D], BF, name="wqkT_bf")
        wv_bf = big.tile([P, D_TILES, H], BF, name="wv_bf")
        nc.scalar.dma_start(wv_bf, wv_dr)
        nc.scalar.dma_start(wqkT_bf, wqkT_dr)

        if _rep == 0 and not zero_bias:
            # small constants, needed from the M1/b1 phases onward
            nc.scalar.dma_start(bd_sb, bd_d.rearrange("(o p) -> p o", p=P))
            nc.scalar.dma_start(bq_col, bq_d.rearrange("(o p) -> p o", p=P))
            for idx, ab in enumerate(aux_rows):
                nc.scalar.dma_start(ab, aux_d[idx:idx + 1, :])

        # ---- G1 = x^T @ Wd  [d, h'] ----
        g1_bf = big.tile([P, D_TILES, H], BF, name="g1_bf")
        for j in range(D_TILES):
            ps1 = psum_acc.tile([P, NF], mybir.dt.float32, name="ps1", tag="acc")
            for a in range(T_TILES):
                nc.tensor.matmul(ps1, lhsT=x_bf[:, a, j * P:(j + 1) * P],
                                 rhs=wd_bf[:, a, :],
                                 start=(a == 0), stop=(a == T_TILES - 1))
            nc.vector.tensor_copy(g1_bf[:, j, :], ps1)

        # ---- X2 = x^T @ x  [d, d'] (symmetric: upper blocks + mirrored) ----
        x2_bf = big.tile([P, D_TILES, D], BF, name="x2_bf")
        for j in range(D_TILES):
            w = D - j * P
            ps2 = psum_acc.tile([P, NF], mybir.dt.float32, name="ps2", tag="acc")
            for a in range(T_TILES):
                nc.tensor.matmul(ps2[:, :w], lhsT=x_bf[:, a, j * P:(j + 1) * P],
                                 rhs=x_bf[:, a, j * P:],
                                 start=(a == 0), stop=(a == T_TILES - 1))
            nc.vector.tensor_copy(x2_bf[:, j, j * P:], ps2[:, :w])
        for j in range(1, D_TILES):
            for jj in range(j):
                ps_tr = psum_tr.tile([P, P], BF, name="ps_tr", tag="tr")
                nc.tensor.transpose(ps_tr, x2_bf[:, jj, j * P:(j + 1) * P], ident)
                nc.vector.tensor_copy(x2_bf[:, j, jj * P:(jj + 1) * P], ps_tr)

        # ---- A2 = Wv^T @ X2 + outer(bv, colsum(x))  [h, d] ----
        a2_bf = big.tile([P, H_TILES, D], BF, name="a2_bf")
        for j in range(H_TILES):
            psb = psum_mm.tile([P, NF], mybir.dt.float32, name="psb", tag="mm")
            for i in range(D_TILES):
                nc.tensor.matmul(psb, lhsT=wv_bf[:, i, j * P:(j + 1) * P],
                                 rhs=x2_bf[:, i, :],
                                 start=(i == 0),
                                 stop=(zero_bias and i == D_TILES - 1))
            if not zero_bias:
                nc.tensor.matmul(psb, lhsT=bv_row[:, j * P:(j + 1) * P],
                                 rhs=sx_row, start=False, stop=True)
            nc.vector.tensor_copy(a2_bf[:, j, :], psb)

        # ---- M1 = (Wq Wk^T) @ G1 + outer(Wq bk, colsum(Wd))  [d, h'] ----
        m1_bf = big.tile([P, D_TILES, H], BF, name="m1_bf")
        for j in range(D_TILES):
            psm = psum_mm.tile([P, NF], mybir.dt.float32, name="psm", tag="mm")
            for i in range(D_TILES):
                nc.tensor.matmul(psm, lhsT=wqkT_bf[:, i, j * P:(j + 1) * P],
                                 rhs=g1_bf[:, i, :],
                                 start=(i == 0),
                                 stop=(zero_bias and i == D_TILES - 1))
            if not zero_bias:
                nc.tensor.matmul(psm, lhsT=w2_row[:, j * P:(j + 1) * P],
                                 rhs=swd_row, start=False, stop=True)
            nc.vector.tensor_copy(m1_bf[:, j, :], psm)

        # ---- b1 = G1^T @ (Wk bq) + bd_eff  [h'] (per-partition cols) ----
        for j in range(H_TILES if not zero_bias else 0):
            psv = psum_mm.tile([P, NF], mybir.dt.float32, name="psv", tag="mm")
            for i in range(D_TILES):
                nc.tensor.matmul(psv[:, 0:1], lhsT=g1_bf[:, i, j * P:(j + 1) * P],
                                 rhs=bq_col[:, i:i + 1],
                                 start=(i == 0), stop=(i == D_TILES - 1))
            nc.scalar.activation(b1_sb[:, j:j + 1], psv[:, 0:1], AF.Identity,
                                 bias=bd_sb[:, j:j + 1], scale=1.0)

        # ---- denseT = sigmoid(M1^T @ xT_own + b1)  [h', s] ----
        dT_bf = big.tile([P, H_TILES, R], BF, name="dT_bf")
        for j in range(H_TILES):
            for c in range(R // NF):
                psd = psum_mm.tile([P, NF], mybir.dt.float32, name="psd", tag="mm")
                for i in range(D_TILES):
                    nc.tensor.matmul(psd, lhsT=m1_bf[:, i, j * P:(j + 1) * P],
                                     rhs=xT_own[:, i, c * NF:(c + 1) * NF],
                                     start=(i == 0), stop=(i == D_TILES - 1))
                nc.scalar.activation(dT_bf[:, j, c * NF:(c + 1) * NF], psd,
                                     AF.Sigmoid,
                                     bias=(0.0 if zero_bias
                                           else b1_sb[:, j:j + 1]), scale=1.0)

        # ---- out = denseT^T @ A2  [s, d] ----
        out_sb = big.tile([P, R_TILES, D], mybir.dt.float32, name="out_sb")
        for m in range(R_TILES):
            pso = psum_mm.tile([P, NF], mybir.dt.float32, name="pso", tag="mm")
            for i in range(H_TILES):
                nc.tensor.matmul(pso, lhsT=dT_bf[:, i, m * P:(m + 1) * P],
                                 rhs=a2_bf[:, i, :],
                                 start=(i == 0), stop=(i == H_TILES - 1))
            if m % 2 == 0:
                nc.vector.tensor_copy(out_sb[:, m, :], pso)
                nc.sync.dma_start(out_dr[m // 4][:, m % 4, :], out_sb[:, m, :])
            else:
                nc.scalar.copy(out_sb[:, m, :], pso)
                nc.scalar.dma_start(out_dr[m // 4][:, m % 4, :], out_sb[:, m, :])

    ctx.close()


def build_nc(repeat=1, zero_bias=False):
    import concourse.mybir as mybir
    import concourse.tile as tile
    from concourse import bacc

    F32 = mybir.dt.float32
    BF = mybir.dt.bfloat16
    nc = bacc.Bacc("TRN2", target_bir_lowering=False, debug=False,
                   num_devices=N_CORES)
    x_d = nc.dram_tensor("x", [S, D], BF, kind="ExternalInput").ap()
    wqkT_d = nc.dram_tensor("wqkT", [D, D], BF, kind="ExternalInput").ap()
    wv_d = nc.dram_tensor("wv", [D, H], BF, kind="ExternalInput").ap()
    wd_d = nc.dram_tensor("wd", [S, H], BF, kind="ExternalInput").ap()
    bq_d = nc.dram_tensor("bq", [H], BF, kind="ExternalInput").ap()
    bd_d = nc.dram_tensor("bd", [H], F32, kind="ExternalInput").ap()
    aux_d = nc.dram_tensor("aux", [4, H], BF, kind="ExternalInput").ap()
    out_d = nc.dram_tensor("out", [R, D], F32, kind="ExternalOutput").ap()

    with tile.TileContext(nc) as tc:
        _build_body(nc, tc, (x_d, wqkT_d, wv_d, wd_d,
                             bq_d, bd_d, aux_d, out_d), repeat=repeat,
                    zero_bias=zero_bias)
    nc.compile()
    return nc


def _get_nc(zero_bias=False):
    if zero_bias not in _NC:
        _NC[zero_bias] = build_nc(zero_bias=zero_bias)
    return _NC[zero_bias]


def make_in_maps(x, Wq, bq, Wk, bk, Wv, bv, Wd, bd):
    bf = ml_dtypes.bfloat16
    x = np.asarray(x, dtype=np.float32)
    Wq = np.asarray(Wq, np.float32)
    Wk = np.asarray(Wk, np.float32)
    bq = np.asarray(bq, np.float32)
    bk = np.asarray(bk, np.float32)
    Wd = np.asarray(Wd, dtype=np.float32)
    x_bf = x.astype(bf)
    wd_bf = np.ascontiguousarray(Wd.astype(bf))
    # host-folded projection products (f32 accurate, then bf16)
    wqkT = np.ascontiguousarray((Wk @ Wq.T).astype(bf))   # (Wq Wk^T)^T
    u = (Wk @ bq)                                          # b1 column
    w2 = (Wq @ bk)                                         # M1 rank-1 row
    swd = Wd.sum(axis=0)
    bd_eff = np.asarray(bd, np.float32) + float(bk @ bq) * swd
    wv_bf = np.asarray(Wv, np.float32).astype(bf)
    in_maps = []
    for core in range(N_CORES):
        b, half = divmod(core, 2)
        r0 = half * R
        xb = np.roll(x_bf[b], -r0, axis=0) if r0 else x_bf[b]
        wd_b = np.roll(wd_bf, -r0, axis=0) if r0 else wd_bf
        aux = np.stack([w2, swd,
                        np.asarray(bv, np.float32),
                        x[b].sum(axis=0)]).astype(bf)
        in_maps.append({
            "x": np.ascontiguousarray(xb),
            "wqkT": wqkT,
            "wv": wv_bf,
            "wd": np.ascontiguousarray(wd_b),
            "bq": u.astype(bf),
            "bd": bd_eff,
            "aux": np.ascontiguousarray(aux),
        })
    return in_maps


def kernel(x, Wq, bq, Wk, bk, Wv, bv, Wd, bd, trace=False):
    global LAST_RESULTS
    from concourse.bass_utils import run_bass_kernel_spmd

    zero_bias = not (np.any(np.asarray(bq)) or np.any(np.asarray(bk))
                     or np.any(np.asarray(bv)) or np.any(np.asarray(bd)))
    nc = _get_nc(zero_bias=zero_bias)
    in_maps = make_in_maps(x, Wq, bq, Wk, bk, Wv, bv, Wd, bd)
    res = run_bass_kernel_spmd(nc, in_maps, core_ids=list(range(N_CORES)),
                               trace=trace)
    LAST_RESULTS = res
    out = np.empty((B, S, D), dtype=np.float32)
    for core in range(N_CORES):
        b, half = divmod(core, 2)
        out[b, half * R:(half + 1) * R] = res.results[core]["out"]
    return out

